# revision 18
# baseline (speedup 1.0000x reference)
"""Causal self-attention (B=4, S=2048, D=1024, H=16, hd=64) on 8 TRN2 cores.

Sharding: core c = (batch b = c//2, head-group g = c%2); each core computes
8 heads for one batch. Out-projection partials are summed on host (the only
cross-shard reduction).

Device kernel layout (all matmul contractions have the contracted dim on
SBUF partitions; everything stays transposed so no on-device transposes):
  qT,kT  [64*2heads, S]  = wqkvT-chunk.T @ xT          (stationary weights)
  v_aug  [S-block, 8*65] = xT-chunk.T @ wvT (+ ones col per head for sums)
  sT     [j 128, i 512]  = kT-slice.T @ qT-slice        (2 heads row-packed,
                           concurrent via 64-row tile_position groups)
  pT     = exp(sT/8)  bf16 via ACT (exp is the ONLY work on ACT; all plain
           copies live on DVE so the ACT queue is pure exp)
  outT   [65, i]        += v_aug.T @ pT   (row 64 accumulates softmax sums)
  attnT  = outT * bcast(1/sums)   (recip reads PSUM directly; one merged
           [1,1024] recip + one [64,1024] partition_broadcast per (pair,ib))
  out    [s 128, e]      = attnT-chunk.T @ woutT-chunk  (accum over c-chunks)

Schedule: input DMAs are issued in consumption order (xt chunk + pair-0
q/k weight cols per d-chunk first, then v cols, then remaining qkv cols,
then wout) and the pair-0 q/k projection runs d-chunk-outer across 8 PSUM
accumulators so the PE starts within the first DMA chunk instead of
waiting for the full 8MB input load.  QKV work for pair p+1 is emitted
between attention i-blocks of pair p so the PE always has independent
matmuls while ACT runs exp.  Causal masks are applied on GPSIMD (SBUF
only) to keep DVE headroom for PSUM drains.
"""
import sys
import os

sys.path.insert(0, "/opt/trn_rl_repo")

import numpy as np
import ml_dtypes
from contextlib import ExitStack

S = 2048
D = 1024
HL = 8          # heads per core
HD = 64
PAIRS = 4       # head pairs per core
NIB = 4         # i-blocks of 512
N_CORES = 8

_CACHE = {}
LAST_EXEC_TIME_NS = None


def _build():
    import concourse.tile as tile
    import concourse.mybir as mybir
    from concourse import bacc

    bf = mybir.dt.bfloat16
    f32 = mybir.dt.float32
    EXP = mybir.ActivationFunctionType.Exp
    GE = mybir.AluOpType.is_ge
    MUL = mybir.AluOpType.mult

    nc = bacc.Bacc("TRN2", target_bir_lowering=False, debug=False,
                   num_devices=N_CORES)
    xT_d = nc.dram_tensor("xT", [D, S], bf, kind="ExternalInput").ap()
    wqkvT_d = nc.dram_tensor("wqkvT", [D, 3 * 512], bf,
                             kind="ExternalInput").ap()
    woutT_d = nc.dram_tensor("woutT", [512, D], bf, kind="ExternalInput").ap()
    out_d = nc.dram_tensor("out", [S, D], f32, kind="ExternalOutput").ap()

    with tile.TileContext(nc) as tc, ExitStack() as ctx:
        sb = ctx.enter_context(tc.tile_pool(name="sb", bufs=1))
        # PSUM: "mm" = 2x [128,1024] (score double-buffer; also the 4 q
        # accumulators during the pair-0 projection), "ps5" = 4x [128,512]
        # (qkv/out-proj accum, AV accum, pair-0 k accums) -> 8 banks.
        mm = ctx.enter_context(tc.tile_pool(name="mm", bufs=2, space="PSUM"))
        ps5 = ctx.enter_context(tc.tile_pool(name="ps5", bufs=4,
                                             space="PSUM"))
        pp = ctx.enter_context(tc.tile_pool(name="pp", bufs=8))
        rsp = ctx.enter_context(tc.tile_pool(name="rsp", bufs=2))
        bcsp = ctx.enter_context(tc.tile_pool(name="bcsp", bufs=2))
        osbp = ctx.enter_context(tc.tile_pool(name="osbp", bufs=3))

        # ---- persistent SBUF tiles -------------------------------------
        xt = [sb.tile([128, S], bf, tag=f"xt{d}", name=f"xt{d}")
              for d in range(8)]
        wqkv = [sb.tile([128, 1536], bf, tag=f"wqkv{d}", name=f"wqkv{d}")
                for d in range(8)]
        wout = [sb.tile([128, D], bf, tag=f"wout{c}", name=f"wout{c}")
                for c in range(4)]
        qT = [sb.tile([128, S], bf, tag=f"qT{p}", name=f"qT{p}")
              for p in range(PAIRS)]
        kT = [sb.tile([128, S], bf, tag=f"kT{p}", name=f"kT{p}")
              for p in range(PAIRS)]
        vaug = [sb.tile([128, HL, HD + 1], bf, tag=f"vaug{s}",
                        name=f"vaug{s}") for s in range(16)]
        attnT = [sb.tile([128, S], bf, tag=f"attnT{p}", name=f"attnT{p}")
                 for p in range(PAIRS)]

        # ---- input DMAs in consumption order ---------------------------
        # per d-chunk: xt + the pair-0 q/k weight columns (feeds the first
        # projection); then v columns (v-phase); then pairs 1-3 q/k; wout.
        # Issue instructions cost ~600ns each on an engine queue, so the
        # startup-critical set is spread across three engines' DGE queues.
        for d in range(8):
            rows = wqkvT_d[128 * d:128 * (d + 1), :]
            nc.sync.dma_start(xt[d][:], xT_d[128 * d:128 * (d + 1), :])
            nc.scalar.dma_start(wqkv[d][:, 0:128], rows[:, 0:128])
            nc.scalar.dma_start(wqkv[d][:, 512:640], rows[:, 512:640])
        for d in range(8):
            rows = wqkvT_d[128 * d:128 * (d + 1), :]
            nc.gpsimd.dma_start(wqkv[d][:, 1024:1536], rows[:, 1024:1536])
        for d in range(8):
            rows = wqkvT_d[128 * d:128 * (d + 1), :]
            nc.sync.dma_start(wqkv[d][:, 128:512], rows[:, 128:512])
            nc.sync.dma_start(wqkv[d][:, 640:1024], rows[:, 640:1024])
        for c in range(4):
            nc.sync.dma_start(wout[c][:], woutT_d[128 * c:128 * (c + 1), :])
        for s in range(16):
            nc.gpsimd.memset(vaug[s][:], 1.0)
        # causal masks for the 4 diagonal offsets: keep where i >= 128*m + j
        masks = [sb.tile([128, 512], bf, tag=f"mask{m}", name=f"mask{m}")
                 for m in range(4)]
        for m in range(4):
            nc.gpsimd.memset(masks[m][:], 1.0)
            nc.gpsimd.affine_select(
                out=masks[m][:], in_=masks[m][:], compare_op=GE, fill=0.0,
                base=-128 * m, channel_multiplier=-1, pattern=[[1, 512]])

        # ---- PE warm-up: dummy matmuls on a never-written SBUF tile run
        # during the input-DMA window (no data deps) and trip HAM to
        # K=8/8 before the first real matmul; their PSUM garbage is
        # overwritten by the projection's start=True. ---------------------
        dumT = sb.tile([128, 512], bf, tag="dumT", name="dumT")
        dumP = mm.tile([128, 1024], f32, tag="mm", name="dumP")
        nc.vector.memset(dumT[:], 0.0)
        for i in range(14):
            nc.tensor.matmul(dumP[:, 0:512], lhsT=dumT[:, 0:128],
                             rhs=dumT[:], start=True, stop=True)

        # ---- pair-0 q/k projection: d-chunk outer over 8 accumulators so
        # the PE starts on the first DMA'd chunk -------------------------
        qacc = [mm.tile([128, 1024], f32, tag="mm", name=f"qacc{i}")
                for i in range(2)]
        kacc = [ps5.tile([128, 512], f32, tag="ps5", name=f"kacc{sc}")
                for sc in range(4)]
        for dc in range(8):
            for sc in range(4):
                nc.tensor.matmul(
                    qacc[sc // 2][:, 512 * (sc % 2):512 * (sc % 2 + 1)],
                    lhsT=wqkv[dc][:, 0:128],
                    rhs=xt[dc][:, 512 * sc:512 * (sc + 1)],
                    start=(dc == 0), stop=(dc == 7))
            for sc in range(4):
                nc.tensor.matmul(
                    kacc[sc][:],
                    lhsT=wqkv[dc][:, 512:640],
                    rhs=xt[dc][:, 512 * sc:512 * (sc + 1)],
                    start=(dc == 0), stop=(dc == 7))
        for sc in range(4):
            nc.vector.tensor_copy(
                qT[0][:, 512 * sc:512 * (sc + 1)],
                qacc[sc // 2][:, 512 * (sc % 2):512 * (sc % 2 + 1)])
            nc.vector.tensor_copy(kT[0][:, 512 * sc:512 * (sc + 1)],
                                  kacc[sc][:])

        # ---- emission helpers ------------------------------------------
        def emit_v(sblk):
            ps = ps5.tile([128, 512], f32, tag="ps5", name=f"vps{sblk}")
            for dc in range(8):
                nc.tensor.matmul(ps[:],
                                 lhsT=xt[dc][:, 128 * sblk:128 * (sblk + 1)],
                                 rhs=wqkv[dc][:, 1024:1536],
                                 start=(dc == 0), stop=(dc == 7))
            nc.vector.tensor_copy(
                vaug[sblk][:, :, 0:64],
                ps[:].rearrange("p (h d) -> p h d", h=HL))

        def emit_qk_chunk(pair, nb, sc):
            # one (nb, sc) accumulation chain; nb < 4 -> q, else k
            dest = qT[pair] if nb < 4 else kT[pair]
            ps = ps5.tile([128, 512], f32, tag="ps5", name=f"qkps{nb}_{sc}")
            for dc in range(8):
                nc.tensor.matmul(
                    ps[:],
                    lhsT=wqkv[dc][:, 128 * nb:128 * (nb + 1)],
                    rhs=xt[dc][:, 512 * sc:512 * (sc + 1)],
                    start=(dc == 0), stop=(dc == 7))
            nc.vector.tensor_copy(dest[:, 512 * sc:512 * (sc + 1)], ps[:])

        def emit_qkexp(pair, ib, jb):
            off = max(0, 128 * (jb - 4 * ib))
            s2 = mm.tile([128, 1024], f32, tag="mm",
                         name=f"s2_{pair}{ib}{jb}")
            for h01 in range(2):
                r0, r1 = 64 * h01, 64 * (h01 + 1)
                nc.tensor.matmul(
                    s2[:, 512 * h01 + off:512 * (h01 + 1)],
                    lhsT=kT[pair][r0:r1, 128 * jb:128 * (jb + 1)],
                    rhs=qT[pair][r0:r1, 512 * ib + off:512 * (ib + 1)],
                    start=True, stop=True)
            pX = pp.tile([128, 1024], bf, tag="pp", name=f"pX{pair}{ib}{jb}")
            s3 = s2[:].rearrange("p (h i) -> p h i", h=2)
            p3 = pX[:].rearrange("p (h i) -> p h i", h=2)
            nc.scalar.activation(p3[:, :, off:512], s3[:, :, off:512],
                                 EXP, scale=0.125)
            if jb >= 4 * ib:
                m = jb - 4 * ib
                nc.vector.tensor_mul(
                    p3[:, :, off:512], p3[:, :, off:512],
                    masks[m][:, off:512].unsqueeze(1).broadcast_to(
                        [128, 2, 512 - off]))
            return pX

        def make_tail(pair, ib, oA, oB):
            # softmax tail, per-head chains: sums rows copied to SBUF on
            # ScE (close to PSUM; custom-DVE recip reading PSUM directly
            # returns garbage on HW), then per-head recip + broadcast +
            # normalize mul into attnT.
            def tail():
                st = rsp.tile([1, 1024], f32, tag="rst",
                              name=f"st{pair}{ib}")
                rs = rsp.tile([1, 1024], f32, tag="rsp",
                              name=f"rs{pair}{ib}")
                bcs = bcsp.tile([64, 1024], f32, tag="bcsp",
                                name=f"bcs{pair}{ib}")
                for h01, oX in ((0, oA), (1, oB)):
                    c0, c1 = 512 * h01, 512 * (h01 + 1)
                    nc.scalar.copy(st[:, c0:c1], oX[64:65, :])
                    nc.vector.reciprocal_approx_fast(rs[:, c0:c1],
                                                     st[:, c0:c1])
                    nc.gpsimd.partition_broadcast(bcs[:, c0:c1],
                                                  rs[:, c0:c1])
                    nc.vector.tensor_mul(
                        attnT[pair][64 * h01:64 * (h01 + 1),
                                    512 * ib:512 * (ib + 1)],
                        oX[0:64, :], bcs[:, c0:c1])
            return tail

        def emit_attn(pair, only_ib, pre_px=None, weave=(),
                      pending=None):
            # Per jb PAIR: both score-pairs + exps first, then the
            # previous ib's deferred softmax tail (so its ScE sums-copies
            # queue up behind this ib's already-ready exps instead of
            # stalling the ACT FIFO), then woven fill work, then the 4 AV
            # matmuls.  The PE queue is strict FIFO, so this keeps
            # independent matmuls between the score matmuls and the
            # exp-dependent AV matmuls.  Returns this ib's tail closure
            # for the caller to pass into the next emit_attn.
            ib = only_ib
            n_jb = 4 * (ib + 1)
            oA = ps5.tile([65, 512], f32, tag="ps5", name=f"oA{pair}{ib}")
            oB = ps5.tile([65, 512], f32, tag="ps5", name=f"oB{pair}{ib}")
            n_pr = n_jb // 2
            # weave groups: (start_t, per_slot, items); per_slot None ->
            # spread evenly over the remaining jb pairs
            groups = []
            for start, per, its in weave:
                its = list(its)
                if per is None:
                    per = -(-len(its) // max(1, n_pr - start))
                groups.append((start, per, its))
            for t in range(n_pr):
                pxs = []
                for jb in (2 * t, 2 * t + 1):
                    if pre_px is not None and jb in pre_px:
                        pxs.append(pre_px[jb])
                    else:
                        pxs.append(emit_qkexp(pair, ib, jb))
                if t == 1 and pending is not None:
                    pending()
                    pending = None
                for start, per, its in groups:
                    if t >= start:
                        idx = (t - start) * per
                        for it in its[idx:idx + per]:
                            it()
                for jb, pX in zip((2 * t, 2 * t + 1), pxs):
                    off = max(0, 128 * (jb - 4 * ib))
                    for h01, oX in ((0, oA), (1, oB)):
                        nc.tensor.matmul(
                            oX[:, off:512],
                            lhsT=vaug[jb][:, 2 * pair + h01, :],
                            rhs=pX[:, 512 * h01 + off:512 * (h01 + 1)],
                            start=(jb == 0), stop=(jb == n_jb - 1))
            if pending is not None:
                pending()
            return make_tail(pair, ib, oA, oB)

        osb_map = {}

        def emit_op_half(sblk, eh):
            if eh == 0:
                osb_map[sblk] = osbp.tile([128, D], f32, tag="osbp",
                                          name=f"osb{sblk}")
            osb = osb_map[sblk]
            ps = ps5.tile([128, 512], f32, tag="ps5", name=f"ops{sblk}{eh}")
            for cc in range(4):
                nc.tensor.matmul(
                    ps[:],
                    lhsT=attnT[cc][:, 128 * sblk:128 * (sblk + 1)],
                    rhs=wout[cc][:, 512 * eh:512 * (eh + 1)],
                    start=(cc == 0), stop=(cc == 3))
            nc.vector.tensor_copy(osb[:, 512 * eh:512 * (eh + 1)], ps[:])
            if eh == 1:
                nc.sync.dma_start(out_d[128 * sblk:128 * (sblk + 1), :],
                                  osb[:])

        # ---- emission order (== scheduler priority): vaug[s] must be
        # written before the attention ib that reads it; attnT before the
        # out-proj s-blocks that read it. exp work starts as early as
        # possible; qk chunks for pair p+1 spread across pair p's ibs;
        # out-proj interleaves with the last pair. -----------------------
        # ib0 of pair 0: QK+exp emitted before the v-phase so ACT starts
        # as early as possible (AV waits for vaug, exp does not)
        def qk_item(pair, chunk):
            # global chunk index 0-7: 0-3 = q cols (nb=pair), 4-7 = k cols
            nb = pair if chunk < 4 else pair + 4
            return lambda: emit_qk_chunk(pair, nb, chunk % 4)

        pre = {jb: emit_qkexp(0, 0, jb) for jb in range(4)}
        for sblk in range(4):
            emit_v(sblk)
        tail = emit_attn(0, only_ib=0, pre_px=pre)
        qk_item(1, 0)()
        qk_item(1, 1)()
        for ib in range(1, NIB):
            # weave this ib's v chunks + 2 qk chunks of pair 1 into the
            # jb-pair loop, front-loaded (2 per pair slot) so the
            # ib-boundary exp-latency bubble is fully filled
            items = ([lambda s=s: emit_v(s) for s in
                      range(4 * ib, 4 * ib + 4)] +
                     [qk_item(1, 2 * ib), qk_item(1, 2 * ib + 1)])
            tail = emit_attn(0, only_ib=ib, weave=[(0, 2, items)],
                             pending=tail)
        for pair in (1, 2):
            for ib in range(NIB):
                items = [qk_item(pair + 1, 2 * ib),
                         qk_item(pair + 1, 2 * ib + 1)]
                tail = emit_attn(pair, only_ib=ib, weave=[(0, 2, items)],
                                 pending=tail)
        # pair 3: out-proj halves of ib k are split 6 + 2: six woven into
        # attn(3, k+1) from the third jb pair (their dependency, the
        # deferred ib-k tail, resolves a couple of blocks in) and two held
        # back as always-ready boundary fill for attn(3, k+2) / the final
        # tail window.
        op_items = [[lambda s=s, e=e: emit_op_half(s, e)
                     for s in range(4 * k, 4 * k + 4) for e in (0, 1)]
                    for k in range(4)]
        tail = emit_attn(3, only_ib=0, pending=tail)
        tail = emit_attn(3, only_ib=1, weave=[(2, None, op_items[0][0:6])],
                         pending=tail)
        tail = emit_attn(3, only_ib=2,
                         weave=[(0, 2, op_items[0][6:8]),
                                (2, None, op_items[1][0:6])],
                         pending=tail)
        tail = emit_attn(3, only_ib=3,
                         weave=[(0, 2, op_items[1][6:8]),
                                (2, None, op_items[2][0:6])],
                         pending=tail)
        tail()
        for it in op_items[2][6:8] + op_items[3]:
            it()

    nc.compile()
    return nc


def _get_nc():
    if "nc" not in _CACHE:
        _CACHE["nc"] = _build()
    return _CACHE["nc"]


def _shard_inputs(x, w_qkv, w_out):
    bf = ml_dtypes.bfloat16
    in_maps = []
    for c in range(N_CORES):
        b, g = divmod(c, 2)
        xT = np.ascontiguousarray(x[b].T).astype(bf)
        wq = w_qkv[512 * g:512 * (g + 1)]
        wk = w_qkv[1024 + 512 * g:1024 + 512 * (g + 1)]
        wv = w_qkv[2048 + 512 * g:2048 + 512 * (g + 1)]
        wqkvT = np.ascontiguousarray(
            np.concatenate([wq, wk, wv], axis=0).T).astype(bf)
        woutT = np.ascontiguousarray(w_out[:, 512 * g:512 * (g + 1)].T
                                     ).astype(bf)
        in_maps.append({"xT": xT, "wqkvT": wqkvT, "woutT": woutT})
    return in_maps


def kernel(x, w_qkv, w_out):
    global LAST_EXEC_TIME_NS
    from concourse.bass_utils import run_bass_kernel_spmd

    nc = _get_nc()
    in_maps = _shard_inputs(np.asarray(x, dtype=np.float32),
                            np.asarray(w_qkv, dtype=np.float32),
                            np.asarray(w_out, dtype=np.float32))
    trace = bool(int(os.environ.get("KBENCH_TRACE", "0")))
    res = run_bass_kernel_spmd(nc, in_maps, list(range(N_CORES)), trace=trace)
    LAST_EXEC_TIME_NS = res.exec_time_ns
    out = np.empty((4, S, D), dtype=np.float32)
    for b in range(4):
        out[b] = res.results[2 * b]["out"] + res.results[2 * b + 1]["out"]
    return out


# revision 21
# speedup vs baseline: 1.0005x; 1.0005x over previous
"""Causal self-attention (B=4, S=2048, D=1024, H=16, hd=64) on 8 TRN2 cores.

Sharding: core c = (batch b = c//2, head-group g = c%2); each core computes
8 heads for one batch. Out-projection partials are summed on host (the only
cross-shard reduction).

Device kernel layout (all matmul contractions have the contracted dim on
SBUF partitions; everything stays transposed so no on-device transposes):
  qT,kT  [64*2heads, S]  = wqkvT-chunk.T @ xT          (stationary weights)
  v_aug  [S-block, 8*65] = xT-chunk.T @ wvT (+ ones col per head for sums)
  sT     [j 128, i 512]  = kT-slice.T @ qT-slice        (2 heads row-packed,
                           concurrent via 64-row tile_position groups)
  pT     = exp(sT/8)  bf16 via ACT (exp is the ONLY work on ACT; all plain
           copies live on DVE so the ACT queue is pure exp)
  outT   [65, i]        += v_aug.T @ pT   (row 64 accumulates softmax sums)
  attnT  = outT * bcast(1/sums)   (recip reads PSUM directly; one merged
           [1,1024] recip + one [64,1024] partition_broadcast per (pair,ib))
  out    [s 128, e]      = attnT-chunk.T @ woutT-chunk  (accum over c-chunks)

Schedule: input DMAs are issued in consumption order (xt chunk + pair-0
q/k weight cols per d-chunk first, then v cols, then remaining qkv cols,
then wout) and the pair-0 q/k projection runs d-chunk-outer across 8 PSUM
accumulators so the PE starts within the first DMA chunk instead of
waiting for the full 8MB input load.  QKV work for pair p+1 is emitted
between attention i-blocks of pair p so the PE always has independent
matmuls while ACT runs exp.  Causal masks are applied on GPSIMD (SBUF
only) to keep DVE headroom for PSUM drains.
"""
import sys
import os

sys.path.insert(0, "/opt/trn_rl_repo")

import numpy as np
import ml_dtypes
from contextlib import ExitStack

S = 2048
D = 1024
HL = 8          # heads per core
HD = 64
PAIRS = 4       # head pairs per core
NIB = 4         # i-blocks of 512
N_CORES = 8

_CACHE = {}
LAST_EXEC_TIME_NS = None


def _build():
    import concourse.tile as tile
    import concourse.mybir as mybir
    from concourse import bacc

    bf = mybir.dt.bfloat16
    fp16 = mybir.dt.float16
    f32 = mybir.dt.float32
    EXP = mybir.ActivationFunctionType.Exp
    GE = mybir.AluOpType.is_ge
    MUL = mybir.AluOpType.mult

    nc = bacc.Bacc("TRN2", target_bir_lowering=False, debug=False,
                   num_devices=N_CORES)
    xT_d = nc.dram_tensor("xT", [D, S], bf, kind="ExternalInput").ap()
    wqkvT_d = nc.dram_tensor("wqkvT", [D, 3 * 512], bf,
                             kind="ExternalInput").ap()
    woutT_d = nc.dram_tensor("woutT", [512, D], bf, kind="ExternalInput").ap()
    out_d = nc.dram_tensor("out", [S, D], fp16, kind="ExternalOutput").ap()

    with tile.TileContext(nc) as tc, ExitStack() as ctx:
        sb = ctx.enter_context(tc.tile_pool(name="sb", bufs=1))
        # PSUM: "mm" = 2x [128,1024] (score double-buffer; also the 4 q
        # accumulators during the pair-0 projection), "ps5" = 4x [128,512]
        # (qkv/out-proj accum, AV accum, pair-0 k accums) -> 8 banks.
        mm = ctx.enter_context(tc.tile_pool(name="mm", bufs=2, space="PSUM"))
        ps5 = ctx.enter_context(tc.tile_pool(name="ps5", bufs=4,
                                             space="PSUM"))
        pp = ctx.enter_context(tc.tile_pool(name="pp", bufs=8))
        rsp = ctx.enter_context(tc.tile_pool(name="rsp", bufs=2))
        bcsp = ctx.enter_context(tc.tile_pool(name="bcsp", bufs=2))
        osbp = ctx.enter_context(tc.tile_pool(name="osbp", bufs=3))

        # ---- persistent SBUF tiles -------------------------------------
        xt = [sb.tile([128, S], bf, tag=f"xt{d}", name=f"xt{d}")
              for d in range(8)]
        wqkv = [sb.tile([128, 1536], bf, tag=f"wqkv{d}", name=f"wqkv{d}")
                for d in range(8)]
        wout = [sb.tile([128, D], bf, tag=f"wout{c}", name=f"wout{c}")
                for c in range(4)]
        qT = [sb.tile([128, S], bf, tag=f"qT{p}", name=f"qT{p}")
              for p in range(PAIRS)]
        kT = [sb.tile([128, S], bf, tag=f"kT{p}", name=f"kT{p}")
              for p in range(PAIRS)]
        vaug = [sb.tile([128, HL, HD + 1], bf, tag=f"vaug{s}",
                        name=f"vaug{s}") for s in range(16)]
        attnT = [sb.tile([128, S], bf, tag=f"attnT{p}", name=f"attnT{p}")
                 for p in range(PAIRS)]

        # ---- input DMAs in consumption order ---------------------------
        # per d-chunk: xt + the pair-0 q/k weight columns (feeds the first
        # projection); then v columns (v-phase); then pairs 1-3 q/k; wout.
        # Issue instructions cost ~600ns each on an engine queue, so the
        # startup-critical set is spread across three engines' DGE queues.
        for d in range(8):
            rows = wqkvT_d[128 * d:128 * (d + 1), :]
            nc.sync.dma_start(xt[d][:], xT_d[128 * d:128 * (d + 1), :])
            nc.scalar.dma_start(wqkv[d][:, 0:128], rows[:, 0:128])
            nc.scalar.dma_start(wqkv[d][:, 512:640], rows[:, 512:640])
        for d in range(8):
            rows = wqkvT_d[128 * d:128 * (d + 1), :]
            nc.gpsimd.dma_start(wqkv[d][:, 1024:1536], rows[:, 1024:1536])
        for d in range(8):
            rows = wqkvT_d[128 * d:128 * (d + 1), :]
            nc.gpsimd.dma_start(wqkv[d][:, 128:512], rows[:, 128:512])
            nc.gpsimd.dma_start(wqkv[d][:, 640:1024], rows[:, 640:1024])
        for c in range(4):
            nc.gpsimd.dma_start(wout[c][:], woutT_d[128 * c:128 * (c + 1), :])
        for s in range(16):
            nc.gpsimd.memset(vaug[s][:], 1.0)
        # causal masks for the 4 diagonal offsets: keep where i >= 128*m + j
        masks = [sb.tile([128, 512], bf, tag=f"mask{m}", name=f"mask{m}")
                 for m in range(4)]
        for m in range(4):
            nc.gpsimd.memset(masks[m][:], 1.0)
            nc.gpsimd.affine_select(
                out=masks[m][:], in_=masks[m][:], compare_op=GE, fill=0.0,
                base=-128 * m, channel_multiplier=-1, pattern=[[1, 512]])

        # ---- pair-0 q/k projection: d-chunk outer over 8 accumulators so
        # the PE starts on the first DMA'd chunk -------------------------
        qacc = [mm.tile([128, 1024], f32, tag="mm", name=f"qacc{i}")
                for i in range(2)]
        kacc = [ps5.tile([128, 512], f32, tag="ps5", name=f"kacc{sc}")
                for sc in range(4)]
        for dc in range(8):
            for sc in range(4):
                nc.tensor.matmul(
                    qacc[sc // 2][:, 512 * (sc % 2):512 * (sc % 2 + 1)],
                    lhsT=wqkv[dc][:, 0:128],
                    rhs=xt[dc][:, 512 * sc:512 * (sc + 1)],
                    start=(dc == 0), stop=(dc == 7))
            for sc in range(4):
                nc.tensor.matmul(
                    kacc[sc][:],
                    lhsT=wqkv[dc][:, 512:640],
                    rhs=xt[dc][:, 512 * sc:512 * (sc + 1)],
                    start=(dc == 0), stop=(dc == 7))
        for sc in range(4):
            nc.vector.tensor_copy(
                qT[0][:, 512 * sc:512 * (sc + 1)],
                qacc[sc // 2][:, 512 * (sc % 2):512 * (sc % 2 + 1)])
            nc.vector.tensor_copy(kT[0][:, 512 * sc:512 * (sc + 1)],
                                  kacc[sc][:])

        # ---- emission helpers ------------------------------------------
        def emit_v(sblk):
            ps = ps5.tile([128, 512], f32, tag="ps5", name=f"vps{sblk}")
            for dc in range(8):
                nc.tensor.matmul(ps[:],
                                 lhsT=xt[dc][:, 128 * sblk:128 * (sblk + 1)],
                                 rhs=wqkv[dc][:, 1024:1536],
                                 start=(dc == 0), stop=(dc == 7))
            nc.vector.tensor_copy(
                vaug[sblk][:, :, 0:64],
                ps[:].rearrange("p (h d) -> p h d", h=HL))

        def emit_qk_chunk(pair, nb, sc):
            # one (nb, sc) accumulation chain; nb < 4 -> q, else k
            dest = qT[pair] if nb < 4 else kT[pair]
            ps = ps5.tile([128, 512], f32, tag="ps5", name=f"qkps{nb}_{sc}")
            for dc in range(8):
                nc.tensor.matmul(
                    ps[:],
                    lhsT=wqkv[dc][:, 128 * nb:128 * (nb + 1)],
                    rhs=xt[dc][:, 512 * sc:512 * (sc + 1)],
                    start=(dc == 0), stop=(dc == 7))
            nc.vector.tensor_copy(dest[:, 512 * sc:512 * (sc + 1)], ps[:])

        def emit_qkexp(pair, ib, jb):
            off = max(0, 128 * (jb - 4 * ib))
            s2 = mm.tile([128, 1024], f32, tag="mm",
                         name=f"s2_{pair}{ib}{jb}")
            for h01 in range(2):
                r0, r1 = 64 * h01, 64 * (h01 + 1)
                nc.tensor.matmul(
                    s2[:, 512 * h01 + off:512 * (h01 + 1)],
                    lhsT=kT[pair][r0:r1, 128 * jb:128 * (jb + 1)],
                    rhs=qT[pair][r0:r1, 512 * ib + off:512 * (ib + 1)],
                    start=True, stop=True)
            pX = pp.tile([128, 1024], bf, tag="pp", name=f"pX{pair}{ib}{jb}")
            s3 = s2[:].rearrange("p (h i) -> p h i", h=2)
            p3 = pX[:].rearrange("p (h i) -> p h i", h=2)
            nc.scalar.activation(p3[:, :, off:512], s3[:, :, off:512],
                                 EXP, scale=0.125)
            if jb >= 4 * ib:
                m = jb - 4 * ib
                nc.vector.tensor_mul(
                    p3[:, :, off:512], p3[:, :, off:512],
                    masks[m][:, off:512].unsqueeze(1).broadcast_to(
                        [128, 2, 512 - off]))
            return pX

        def make_tail(pair, ib, oA, oB):
            # softmax tail, per-head chains: sums rows copied to SBUF on
            # ScE (close to PSUM; custom-DVE recip reading PSUM directly
            # returns garbage on HW), then per-head recip + broadcast +
            # normalize mul into attnT.
            def tail():
                st = rsp.tile([1, 1024], f32, tag="rst",
                              name=f"st{pair}{ib}")
                rs = rsp.tile([1, 1024], f32, tag="rsp",
                              name=f"rs{pair}{ib}")
                bcs = bcsp.tile([64, 1024], f32, tag="bcsp",
                                name=f"bcs{pair}{ib}")
                for h01, oX in ((0, oA), (1, oB)):
                    c0, c1 = 512 * h01, 512 * (h01 + 1)
                    nc.scalar.copy(st[:, c0:c1], oX[64:65, :])
                    nc.vector.reciprocal_approx_fast(rs[:, c0:c1],
                                                     st[:, c0:c1])
                    nc.gpsimd.partition_broadcast(bcs[:, c0:c1],
                                                  rs[:, c0:c1])
                    nc.vector.tensor_mul(
                        attnT[pair][64 * h01:64 * (h01 + 1),
                                    512 * ib:512 * (ib + 1)],
                        oX[0:64, :], bcs[:, c0:c1])
            return tail

        def emit_attn(pair, only_ib, pre_px=None, weave=(),
                      pending=None):
            # Per jb PAIR: both score-pairs + exps first, then the
            # previous ib's deferred softmax tail (so its ScE sums-copies
            # queue up behind this ib's already-ready exps instead of
            # stalling the ACT FIFO), then woven fill work, then the 4 AV
            # matmuls.  The PE queue is strict FIFO, so this keeps
            # independent matmuls between the score matmuls and the
            # exp-dependent AV matmuls.  Returns this ib's tail closure
            # for the caller to pass into the next emit_attn.
            ib = only_ib
            n_jb = 4 * (ib + 1)
            oA = ps5.tile([65, 512], f32, tag="ps5", name=f"oA{pair}{ib}")
            oB = ps5.tile([65, 512], f32, tag="ps5", name=f"oB{pair}{ib}")
            n_pr = n_jb // 2
            # weave groups: (start_t, per_slot, items); per_slot None ->
            # spread evenly over the remaining jb pairs
            groups = []
            for start, per, its in weave:
                its = list(its)
                if per is None:
                    per = -(-len(its) // max(1, n_pr - start))
                groups.append((start, per, its))
            for t in range(n_pr):
                pxs = []
                for jb in (2 * t, 2 * t + 1):
                    if pre_px is not None and jb in pre_px:
                        pxs.append(pre_px[jb])
                    else:
                        pxs.append(emit_qkexp(pair, ib, jb))
                if t == 1 and pending is not None:
                    pending()
                    pending = None
                for start, per, its in groups:
                    if t >= start:
                        idx = (t - start) * per
                        for it in its[idx:idx + per]:
                            it()
                for jb, pX in zip((2 * t, 2 * t + 1), pxs):
                    off = max(0, 128 * (jb - 4 * ib))
                    for h01, oX in ((0, oA), (1, oB)):
                        nc.tensor.matmul(
                            oX[:, off:512],
                            lhsT=vaug[jb][:, 2 * pair + h01, :],
                            rhs=pX[:, 512 * h01 + off:512 * (h01 + 1)],
                            start=(jb == 0), stop=(jb == n_jb - 1))
            if pending is not None:
                pending()
            return make_tail(pair, ib, oA, oB)

        osb_map = {}

        def emit_op_half(sblk, eh):
            if eh == 0:
                osb_map[sblk] = osbp.tile([128, D], fp16, tag="osbp",
                                          name=f"osb{sblk}")
            osb = osb_map[sblk]
            ps = ps5.tile([128, 512], f32, tag="ps5", name=f"ops{sblk}{eh}")
            for cc in range(4):
                nc.tensor.matmul(
                    ps[:],
                    lhsT=attnT[cc][:, 128 * sblk:128 * (sblk + 1)],
                    rhs=wout[cc][:, 512 * eh:512 * (eh + 1)],
                    start=(cc == 0), stop=(cc == 3))
            nc.vector.tensor_copy(osb[:, 512 * eh:512 * (eh + 1)], ps[:])
            if eh == 1:
                nc.sync.dma_start(out_d[128 * sblk:128 * (sblk + 1), :],
                                  osb[:])

        # ---- emission order (== scheduler priority): vaug[s] must be
        # written before the attention ib that reads it; attnT before the
        # out-proj s-blocks that read it. exp work starts as early as
        # possible; qk chunks for pair p+1 spread across pair p's ibs;
        # out-proj interleaves with the last pair. -----------------------
        # ib0 of pair 0: QK+exp emitted before the v-phase so ACT starts
        # as early as possible (AV waits for vaug, exp does not)
        def qk_item(pair, chunk):
            # global chunk index 0-7: 0-3 = q cols (nb=pair), 4-7 = k cols
            nb = pair if chunk < 4 else pair + 4
            return lambda: emit_qk_chunk(pair, nb, chunk % 4)

        pre = {jb: emit_qkexp(0, 0, jb) for jb in range(4)}
        for sblk in range(4):
            emit_v(sblk)
        tail = emit_attn(0, only_ib=0, pre_px=pre)
        qk_item(1, 0)()
        qk_item(1, 1)()
        for ib in range(1, NIB):
            # weave this ib's v chunks + 2 qk chunks of pair 1 into the
            # jb-pair loop, front-loaded (2 per pair slot) so the
            # ib-boundary exp-latency bubble is fully filled
            items = ([lambda s=s: emit_v(s) for s in
                      range(4 * ib, 4 * ib + 4)] +
                     [qk_item(1, 2 * ib), qk_item(1, 2 * ib + 1)])
            tail = emit_attn(0, only_ib=ib, weave=[(0, 2, items)],
                             pending=tail)
        for pair in (1, 2):
            for ib in range(NIB):
                items = [qk_item(pair + 1, 2 * ib),
                         qk_item(pair + 1, 2 * ib + 1)]
                tail = emit_attn(pair, only_ib=ib, weave=[(0, 2, items)],
                                 pending=tail)
        # pair 3: out-proj halves of ib k are split 6 + 2: six woven into
        # attn(3, k+1) from the third jb pair (their dependency, the
        # deferred ib-k tail, resolves a couple of blocks in) and two held
        # back as always-ready boundary fill for attn(3, k+2) / the final
        # tail window.
        op_items = [[lambda s=s, e=e: emit_op_half(s, e)
                     for s in range(4 * k, 4 * k + 4) for e in (0, 1)]
                    for k in range(4)]
        tail = emit_attn(3, only_ib=0, pending=tail)
        tail = emit_attn(3, only_ib=1, weave=[(2, None, op_items[0][0:6])],
                         pending=tail)
        tail = emit_attn(3, only_ib=2,
                         weave=[(0, 2, op_items[0][6:8]),
                                (2, None, op_items[1][0:6])],
                         pending=tail)
        tail = emit_attn(3, only_ib=3,
                         weave=[(0, 2, op_items[1][6:8]),
                                (2, None, op_items[2][0:6])],
                         pending=tail)
        tail()
        for it in op_items[2][6:8] + op_items[3]:
            it()

    nc.compile()
    return nc


def _get_nc():
    if "nc" not in _CACHE:
        _CACHE["nc"] = _build()
    return _CACHE["nc"]


def _shard_inputs(x, w_qkv, w_out):
    bf = ml_dtypes.bfloat16
    in_maps = []
    for c in range(N_CORES):
        b, g = divmod(c, 2)
        xT = np.ascontiguousarray(x[b].T).astype(bf)
        wq = w_qkv[512 * g:512 * (g + 1)]
        wk = w_qkv[1024 + 512 * g:1024 + 512 * (g + 1)]
        wv = w_qkv[2048 + 512 * g:2048 + 512 * (g + 1)]
        wqkvT = np.ascontiguousarray(
            np.concatenate([wq, wk, wv], axis=0).T).astype(bf)
        woutT = np.ascontiguousarray(w_out[:, 512 * g:512 * (g + 1)].T
                                     ).astype(bf)
        in_maps.append({"xT": xT, "wqkvT": wqkvT, "woutT": woutT})
    return in_maps


def kernel(x, w_qkv, w_out):
    global LAST_EXEC_TIME_NS
    from concourse.bass_utils import run_bass_kernel_spmd

    nc = _get_nc()
    in_maps = _shard_inputs(np.asarray(x, dtype=np.float32),
                            np.asarray(w_qkv, dtype=np.float32),
                            np.asarray(w_out, dtype=np.float32))
    trace = bool(int(os.environ.get("KBENCH_TRACE", "0")))
    res = run_bass_kernel_spmd(nc, in_maps, list(range(N_CORES)), trace=trace)
    LAST_EXEC_TIME_NS = res.exec_time_ns
    out = np.empty((4, S, D), dtype=np.float32)
    for b in range(4):
        out[b] = (res.results[2 * b]["out"].astype(np.float32) +
                  res.results[2 * b + 1]["out"].astype(np.float32))
    return out


# revision 23
# speedup vs baseline: 1.0321x; 1.0316x over previous
"""Causal self-attention (B=4, S=2048, D=1024, H=16, hd=64) on 8 TRN2 cores.

Sharding: core c = (batch b = c//2, head-group g = c%2); each core computes
8 heads for one batch. Out-projection partials are summed on host (the only
cross-shard reduction).

Device kernel layout (all matmul contractions have the contracted dim on
SBUF partitions; everything stays transposed so no on-device transposes):
  qT,kT  [64*2heads, S]  = wqkvT-chunk.T @ xT          (stationary weights)
  v_aug  [S-block, 8*65] = xT-chunk.T @ wvT (+ ones col per head for sums)
  sT     [j 128, i 512]  = kT-slice.T @ qT-slice        (2 heads row-packed,
                           concurrent via 64-row tile_position groups)
  pT     = exp(sT/8)  bf16 via ACT (exp is the ONLY work on ACT; all plain
           copies live on DVE so the ACT queue is pure exp)
  outT   [65, i]        += v_aug.T @ pT   (row 64 accumulates softmax sums)
  attnT  = outT * bcast(1/sums)   (recip reads PSUM directly; one merged
           [1,1024] recip + one [64,1024] partition_broadcast per (pair,ib))
  out    [s 128, e]      = attnT-chunk.T @ woutT-chunk  (accum over c-chunks)

Schedule: input DMAs are issued in consumption order (xt chunk + pair-0
q/k weight cols per d-chunk first, then v cols, then remaining qkv cols,
then wout) and the pair-0 q/k projection runs d-chunk-outer across 8 PSUM
accumulators so the PE starts within the first DMA chunk instead of
waiting for the full 8MB input load.  QKV work for pair p+1 is emitted
between attention i-blocks of pair p so the PE always has independent
matmuls while ACT runs exp.  Causal masks are applied on GPSIMD (SBUF
only) to keep DVE headroom for PSUM drains.
"""
import sys
import os

sys.path.insert(0, "/opt/trn_rl_repo")

import numpy as np
import ml_dtypes
from contextlib import ExitStack

S = 2048
D = 1024
HL = 8          # heads per core
HD = 64
PAIRS = 4       # head pairs per core
NIB = 4         # i-blocks of 512
N_CORES = 8

_CACHE = {}
LAST_EXEC_TIME_NS = None


def _build():
    import concourse.tile as tile
    import concourse.mybir as mybir
    from concourse import bacc

    bf = mybir.dt.bfloat16
    fp16 = mybir.dt.float16
    f32 = mybir.dt.float32
    EXP = mybir.ActivationFunctionType.Exp
    GE = mybir.AluOpType.is_ge
    MUL = mybir.AluOpType.mult

    nc = bacc.Bacc("TRN2", target_bir_lowering=False, debug=False,
                   num_devices=N_CORES)
    xT_d = nc.dram_tensor("xT", [D, S], bf, kind="ExternalInput").ap()
    wqkvT_d = nc.dram_tensor("wqkvT", [D, 3 * 512], bf,
                             kind="ExternalInput").ap()
    woutT_d = nc.dram_tensor("woutT", [512, D], bf, kind="ExternalInput").ap()
    out_d = nc.dram_tensor("out", [S, D], fp16, kind="ExternalOutput").ap()

    with tile.TileContext(nc) as tc, ExitStack() as ctx:
        sb = ctx.enter_context(tc.tile_pool(name="sb", bufs=1))
        # PSUM: "mm" = 2x [128,1024] (score double-buffer; also the 4 q
        # accumulators during the pair-0 projection), "ps5" = 4x [128,512]
        # (qkv/out-proj accum, AV accum, pair-0 k accums) -> 8 banks.
        mm = ctx.enter_context(tc.tile_pool(name="mm", bufs=2, space="PSUM"))
        ps5 = ctx.enter_context(tc.tile_pool(name="ps5", bufs=4,
                                             space="PSUM"))
        pp = ctx.enter_context(tc.tile_pool(name="pp", bufs=8))
        rsp = ctx.enter_context(tc.tile_pool(name="rsp", bufs=2))
        bcsp = ctx.enter_context(tc.tile_pool(name="bcsp", bufs=2))
        osbp = ctx.enter_context(tc.tile_pool(name="osbp", bufs=3))

        # ---- persistent SBUF tiles -------------------------------------
        xt = [sb.tile([128, S], bf, tag=f"xt{d}", name=f"xt{d}")
              for d in range(8)]
        wqkv = [sb.tile([128, 1536], bf, tag=f"wqkv{d}", name=f"wqkv{d}")
                for d in range(8)]
        wout = [sb.tile([128, D], bf, tag=f"wout{c}", name=f"wout{c}")
                for c in range(4)]
        qT = [sb.tile([128, S], bf, tag=f"qT{p}", name=f"qT{p}")
              for p in range(PAIRS)]
        kT = [sb.tile([128, S], bf, tag=f"kT{p}", name=f"kT{p}")
              for p in range(PAIRS)]
        vaug = [sb.tile([128, HL, HD + 1], bf, tag=f"vaug{s}",
                        name=f"vaug{s}") for s in range(16)]
        attnT = [sb.tile([128, S], bf, tag=f"attnT{p}", name=f"attnT{p}")
                 for p in range(PAIRS)]

        # ---- input DMAs in consumption order ---------------------------
        # per d-chunk: xt + the pair-0 q/k weight columns (feeds the first
        # projection); then v columns (v-phase); then pairs 1-3 q/k; wout.
        # Issue instructions cost ~600ns each on an engine queue, so the
        # startup-critical set is spread across three engines' DGE queues.
        for d in range(8):
            rows = wqkvT_d[128 * d:128 * (d + 1), :]
            nc.sync.dma_start(xt[d][:], xT_d[128 * d:128 * (d + 1), :])
            nc.sync.dma_start(wqkv[d][:, 0:128], rows[:, 0:128])
            nc.sync.dma_start(wqkv[d][:, 512:640], rows[:, 512:640])
        for d in range(8):
            rows = wqkvT_d[128 * d:128 * (d + 1), :]
            nc.sync.dma_start(wqkv[d][:, 1024:1536], rows[:, 1024:1536])
        for d in range(8):
            rows = wqkvT_d[128 * d:128 * (d + 1), :]
            nc.sync.dma_start(wqkv[d][:, 128:512], rows[:, 128:512])
            nc.sync.dma_start(wqkv[d][:, 640:1024], rows[:, 640:1024])
        for c in range(4):
            nc.sync.dma_start(wout[c][:], woutT_d[128 * c:128 * (c + 1), :])
        for s in range(16):
            nc.gpsimd.memset(vaug[s][:], 1.0)
        # causal masks for the 4 diagonal offsets: keep where i >= 128*m + j
        masks = [sb.tile([128, 512], bf, tag=f"mask{m}", name=f"mask{m}")
                 for m in range(4)]
        for m in range(4):
            nc.gpsimd.memset(masks[m][:], 1.0)
            nc.gpsimd.affine_select(
                out=masks[m][:], in_=masks[m][:], compare_op=GE, fill=0.0,
                base=-128 * m, channel_multiplier=-1, pattern=[[1, 512]])

        # ---- pair-0 q/k projection: d-chunk outer over 8 accumulators so
        # the PE starts on the first DMA'd chunk -------------------------
        qacc = [mm.tile([128, 1024], f32, tag="mm", name=f"qacc{i}")
                for i in range(2)]
        kacc = [ps5.tile([128, 512], f32, tag="ps5", name=f"kacc{sc}")
                for sc in range(4)]
        for dc in range(8):
            for sc in range(4):
                nc.tensor.matmul(
                    qacc[sc // 2][:, 512 * (sc % 2):512 * (sc % 2 + 1)],
                    lhsT=wqkv[dc][:, 0:128],
                    rhs=xt[dc][:, 512 * sc:512 * (sc + 1)],
                    start=(dc == 0), stop=(dc == 7))
            for sc in range(4):
                nc.tensor.matmul(
                    kacc[sc][:],
                    lhsT=wqkv[dc][:, 512:640],
                    rhs=xt[dc][:, 512 * sc:512 * (sc + 1)],
                    start=(dc == 0), stop=(dc == 7))
        for sc in range(4):
            nc.vector.tensor_copy(
                qT[0][:, 512 * sc:512 * (sc + 1)],
                qacc[sc // 2][:, 512 * (sc % 2):512 * (sc % 2 + 1)])
            nc.vector.tensor_copy(kT[0][:, 512 * sc:512 * (sc + 1)],
                                  kacc[sc][:])

        # ---- emission helpers ------------------------------------------
        def emit_v(sblk):
            ps = ps5.tile([128, 512], f32, tag="ps5", name=f"vps{sblk}")
            for dc in range(8):
                nc.tensor.matmul(ps[:],
                                 lhsT=xt[dc][:, 128 * sblk:128 * (sblk + 1)],
                                 rhs=wqkv[dc][:, 1024:1536],
                                 start=(dc == 0), stop=(dc == 7))
            nc.vector.tensor_copy(
                vaug[sblk][:, :, 0:64],
                ps[:].rearrange("p (h d) -> p h d", h=HL))

        def emit_qk_chunk(pair, nb, sc):
            # one (nb, sc) accumulation chain; nb < 4 -> q, else k
            dest = qT[pair] if nb < 4 else kT[pair]
            ps = ps5.tile([128, 512], f32, tag="ps5", name=f"qkps{nb}_{sc}")
            for dc in range(8):
                nc.tensor.matmul(
                    ps[:],
                    lhsT=wqkv[dc][:, 128 * nb:128 * (nb + 1)],
                    rhs=xt[dc][:, 512 * sc:512 * (sc + 1)],
                    start=(dc == 0), stop=(dc == 7))
            nc.vector.tensor_copy(dest[:, 512 * sc:512 * (sc + 1)], ps[:])

        def emit_qkexp(pair, ib, jb):
            off = max(0, 128 * (jb - 4 * ib))
            s2 = mm.tile([128, 1024], f32, tag="mm",
                         name=f"s2_{pair}{ib}{jb}")
            for h01 in range(2):
                r0, r1 = 64 * h01, 64 * (h01 + 1)
                nc.tensor.matmul(
                    s2[:, 512 * h01 + off:512 * (h01 + 1)],
                    lhsT=kT[pair][r0:r1, 128 * jb:128 * (jb + 1)],
                    rhs=qT[pair][r0:r1, 512 * ib + off:512 * (ib + 1)],
                    start=True, stop=True)
            pX = pp.tile([128, 1024], bf, tag="pp", name=f"pX{pair}{ib}{jb}")
            s3 = s2[:].rearrange("p (h i) -> p h i", h=2)
            p3 = pX[:].rearrange("p (h i) -> p h i", h=2)
            nc.scalar.activation(p3[:, :, off:512], s3[:, :, off:512],
                                 EXP, scale=0.125)
            if jb >= 4 * ib:
                m = jb - 4 * ib
                nc.vector.tensor_mul(
                    p3[:, :, off:512], p3[:, :, off:512],
                    masks[m][:, off:512].unsqueeze(1).broadcast_to(
                        [128, 2, 512 - off]))
            return pX

        def make_tail(pair, ib, oA, oB):
            # softmax tail, per-head chains: sums rows copied to SBUF on
            # ScE (close to PSUM; custom-DVE recip reading PSUM directly
            # returns garbage on HW), then per-head recip + broadcast +
            # normalize mul into attnT.
            def tail():
                st = rsp.tile([1, 1024], f32, tag="rst",
                              name=f"st{pair}{ib}")
                rs = rsp.tile([1, 1024], f32, tag="rsp",
                              name=f"rs{pair}{ib}")
                bcs = bcsp.tile([64, 1024], f32, tag="bcsp",
                                name=f"bcs{pair}{ib}")
                for h01, oX in ((0, oA), (1, oB)):
                    c0, c1 = 512 * h01, 512 * (h01 + 1)
                    nc.scalar.copy(st[:, c0:c1], oX[64:65, :])
                    nc.vector.reciprocal_approx_fast(rs[:, c0:c1],
                                                     st[:, c0:c1])
                    nc.gpsimd.partition_broadcast(bcs[:, c0:c1],
                                                  rs[:, c0:c1])
                    nc.vector.tensor_mul(
                        attnT[pair][64 * h01:64 * (h01 + 1),
                                    512 * ib:512 * (ib + 1)],
                        oX[0:64, :], bcs[:, c0:c1])
            return tail

        def emit_attn(pair, only_ib, pre_px=None, weave=(),
                      pending=None):
            # Per jb PAIR: both score-pairs + exps first, then the
            # previous ib's deferred softmax tail (so its ScE sums-copies
            # queue up behind this ib's already-ready exps instead of
            # stalling the ACT FIFO), then woven fill work, then the 4 AV
            # matmuls.  The PE queue is strict FIFO, so this keeps
            # independent matmuls between the score matmuls and the
            # exp-dependent AV matmuls.  Returns this ib's tail closure
            # for the caller to pass into the next emit_attn.
            ib = only_ib
            n_jb = 4 * (ib + 1)
            oA = ps5.tile([65, 512], f32, tag="ps5", name=f"oA{pair}{ib}")
            oB = ps5.tile([65, 512], f32, tag="ps5", name=f"oB{pair}{ib}")
            n_pr = n_jb // 2
            # weave groups: (start_t, per_slot, items); per_slot None ->
            # spread evenly over the remaining jb pairs
            groups = []
            for start, per, its in weave:
                its = list(its)
                if per is None:
                    per = -(-len(its) // max(1, n_pr - start))
                groups.append((start, per, its))
            for t in range(n_pr):
                pxs = []
                for jb in (2 * t, 2 * t + 1):
                    if pre_px is not None and jb in pre_px:
                        pxs.append(pre_px[jb])
                    else:
                        pxs.append(emit_qkexp(pair, ib, jb))
                if t == 1 and pending is not None:
                    pending()
                    pending = None
                for start, per, its in groups:
                    if t >= start:
                        idx = (t - start) * per
                        for it in its[idx:idx + per]:
                            it()
                for jb, pX in zip((2 * t, 2 * t + 1), pxs):
                    off = max(0, 128 * (jb - 4 * ib))
                    for h01, oX in ((0, oA), (1, oB)):
                        nc.tensor.matmul(
                            oX[:, off:512],
                            lhsT=vaug[jb][:, 2 * pair + h01, :],
                            rhs=pX[:, 512 * h01 + off:512 * (h01 + 1)],
                            start=(jb == 0), stop=(jb == n_jb - 1))
            if pending is not None:
                pending()
            return make_tail(pair, ib, oA, oB)

        osb_map = {}

        def emit_op_half(sblk, eh):
            if eh == 0:
                osb_map[sblk] = osbp.tile([128, D], fp16, tag="osbp",
                                          name=f"osb{sblk}")
            osb = osb_map[sblk]
            ps = ps5.tile([128, 512], f32, tag="ps5", name=f"ops{sblk}{eh}")
            for cc in range(4):
                nc.tensor.matmul(
                    ps[:],
                    lhsT=attnT[cc][:, 128 * sblk:128 * (sblk + 1)],
                    rhs=wout[cc][:, 512 * eh:512 * (eh + 1)],
                    start=(cc == 0), stop=(cc == 3))
            nc.vector.tensor_copy(osb[:, 512 * eh:512 * (eh + 1)], ps[:])
            if eh == 1:
                nc.sync.dma_start(out_d[128 * sblk:128 * (sblk + 1), :],
                                  osb[:])

        # ---- emission order (== scheduler priority): vaug[s] must be
        # written before the attention ib that reads it; attnT before the
        # out-proj s-blocks that read it. exp work starts as early as
        # possible; qk chunks for pair p+1 spread across pair p's ibs;
        # out-proj interleaves with the last pair. -----------------------
        # ib0 of pair 0: QK+exp emitted before the v-phase so ACT starts
        # as early as possible (AV waits for vaug, exp does not)
        def qk_item(pair, chunk):
            # global chunk index 0-7: 0-3 = q cols (nb=pair), 4-7 = k cols
            nb = pair if chunk < 4 else pair + 4
            return lambda: emit_qk_chunk(pair, nb, chunk % 4)

        pre = {jb: emit_qkexp(0, 0, jb) for jb in range(4)}
        for sblk in range(4):
            emit_v(sblk)
        # qk-chunk boundary allocation: chunk sc of pair p (q: 0-3, k:
        # 4-7) is first consumed by attn(p, sc%4), so a pair's early
        # chunks weave into the PREVIOUS pair's ibs and its later chunks
        # into its own early ibs — every ib boundary gets always-ready PE
        # fill for the exp-latency bubble.
        chunk_at = {
            (0, 2): [qk_item(1, 0)], (0, 3): [qk_item(1, 4)],
            (1, 0): [qk_item(1, 1), qk_item(1, 5)],
            (1, 1): [qk_item(1, 2), qk_item(1, 6)],
            (1, 2): [qk_item(1, 3), qk_item(1, 7)],
            (1, 3): [qk_item(2, 0), qk_item(2, 4)],
            (2, 0): [qk_item(2, 1), qk_item(2, 5)],
            (2, 1): [qk_item(2, 2), qk_item(2, 6)],
            (2, 2): [qk_item(2, 3), qk_item(2, 7)],
            (2, 3): [qk_item(3, 0), qk_item(3, 4)],
            (3, 0): [qk_item(3, 1), qk_item(3, 5)],
            (3, 1): [qk_item(3, 2), qk_item(3, 6)],
            (3, 2): [qk_item(3, 3), qk_item(3, 7)],
        }
        op_items = [[lambda s=s, e=e: emit_op_half(s, e)
                     for s in range(4 * k, 4 * k + 4) for e in (0, 1)]
                    for k in range(4)]

        tail = emit_attn(0, only_ib=0, pre_px=pre)
        for ib in range(1, NIB):
            items = ([lambda s=s: emit_v(s) for s in
                      range(4 * ib, 4 * ib + 4)] +
                     chunk_at.get((0, ib), []))
            tail = emit_attn(0, only_ib=ib, weave=[(0, 2, items)],
                             pending=tail)
        for pair in (1, 2):
            for ib in range(NIB):
                tail = emit_attn(pair, only_ib=ib,
                                 weave=[(0, 2, chunk_at[(pair, ib)])],
                                 pending=tail)
        # pair 3: out-proj halves of ib k are split 6 + 2: six woven into
        # attn(3, k+1) from the third jb pair (their dependency, the
        # deferred ib-k tail, resolves a couple of blocks in) and two held
        # back as always-ready boundary fill later / the final tail window.
        tail = emit_attn(3, only_ib=0,
                         weave=[(0, 2, chunk_at[(3, 0)])], pending=tail)
        tail = emit_attn(3, only_ib=1,
                         weave=[(0, 2, chunk_at[(3, 1)]),
                                (2, None, op_items[0][0:6])],
                         pending=tail)
        tail = emit_attn(3, only_ib=2,
                         weave=[(0, 2, chunk_at[(3, 2)]),
                                (1, 2, op_items[0][6:8]),
                                (2, None, op_items[1][0:6])],
                         pending=tail)
        tail = emit_attn(3, only_ib=3,
                         weave=[(0, 2, op_items[1][6:8]),
                                (2, None, op_items[2][0:6])],
                         pending=tail)
        tail()
        for it in op_items[2][6:8] + op_items[3]:
            it()

    nc.compile()
    return nc


def _get_nc():
    if "nc" not in _CACHE:
        _CACHE["nc"] = _build()
    return _CACHE["nc"]


def _shard_inputs(x, w_qkv, w_out):
    bf = ml_dtypes.bfloat16
    in_maps = []
    for c in range(N_CORES):
        b, g = divmod(c, 2)
        xT = np.ascontiguousarray(x[b].T).astype(bf)
        wq = w_qkv[512 * g:512 * (g + 1)]
        wk = w_qkv[1024 + 512 * g:1024 + 512 * (g + 1)]
        wv = w_qkv[2048 + 512 * g:2048 + 512 * (g + 1)]
        wqkvT = np.ascontiguousarray(
            np.concatenate([wq, wk, wv], axis=0).T).astype(bf)
        woutT = np.ascontiguousarray(w_out[:, 512 * g:512 * (g + 1)].T
                                     ).astype(bf)
        in_maps.append({"xT": xT, "wqkvT": wqkvT, "woutT": woutT})
    return in_maps


def kernel(x, w_qkv, w_out):
    global LAST_EXEC_TIME_NS
    from concourse.bass_utils import run_bass_kernel_spmd

    nc = _get_nc()
    in_maps = _shard_inputs(np.asarray(x, dtype=np.float32),
                            np.asarray(w_qkv, dtype=np.float32),
                            np.asarray(w_out, dtype=np.float32))
    trace = bool(int(os.environ.get("KBENCH_TRACE", "0")))
    res = run_bass_kernel_spmd(nc, in_maps, list(range(N_CORES)), trace=trace)
    LAST_EXEC_TIME_NS = res.exec_time_ns
    out = np.empty((4, S, D), dtype=np.float32)
    for b in range(4):
        out[b] = (res.results[2 * b]["out"].astype(np.float32) +
                  res.results[2 * b + 1]["out"].astype(np.float32))
    return out
